# revision 1
# baseline (speedup 1.0000x reference)
"""Trainium2 Bass kernel for bidirectional cross-attention (nn_CrossAttention).

Reference computation (per batch b, N=1024 tokens, D=768 dims):
    sim1  = image1 @ image2^T            [N, N]
    out2  = l2norm(softmax(sim1) @ image2) + 2*image2
    sim2  = image2 @ image1^T
    out1  = l2norm(softmax(sim2) @ image1) + 2*image1

Key algebraic simplification: l2norm(softmax(S) @ V) == l2norm(exp(S - rowmax) @ V)
because the softmax denominator is a positive per-row scalar that the L2
normalization cancels.  So the kernel never computes the softmax sum.

Sharding: pure data parallel, B=16 batches -> 2 per core across 8 cores.

Per-core pipeline (matmuls in bf16, accumulation + epilogue in fp32):
  - SWDGE cast-DMA loads images as bf16 natural chunk tiles [128, 768]
  - PE transposes (identity matmul) build the [D, N] layout [128, 6, 1024]
  - matmul1: S[q,:] = Q^T.T @ K^T  (6 accumulating matmuls x 2 PSUM banks)
  - softmax:  -rowmax on DVE, exp via ACT (PSUM -> bf16 SBUF)
  - PE transposes P -> P^T [128, 8, 128] (PSUM), ACT copies to SBUF
  - matmul2: O = P^T.T @ V (8 accumulating matmuls x 2 banks)
  - epilogue: ss4 = sum((2O)^2) (ACT Square+accum), s = sqrt (ACT),
              inv = 1/s (DVE), T1 = O * inv (ACT, = O / (2*||O||)),
              out = (T1 + image_kv) * 2 (DVE)  == l2norm(O) + 2*image_kv

The three PE stages are software-pipelined (mm1(i) | ptrans(i-1) | mm2(i-2))
so the PE never waits on the softmax chain of the same iteration.

All regular DMA goes through SWDGE (gpsimd); DMA-transpose (xbar) is not used
at all because the XPOSE instruction only encodes a single sync wait, which
Tile's vector-clock closure overflows for any mid-chain transpose.
"""

import os
import sys

import numpy as np

for _p in ("/opt/trn_rl_repo", "/root/.axon_site/_ro/trn_rl_repo"):
    if os.path.isdir(_p) and _p not in sys.path:
        sys.path.append(_p)

B, N, D = 16, 1024, 768
NCORES = 8
BPC = B // NCORES  # batches per core
P = 128
NT = N // P  # 8 token chunks
DT = D // P  # 6 feature chunks

_PROGRAM_CACHE = {}


def build_program():
    """Build the per-core Bass program (SPMD: identical on all cores)."""
    import concourse.mybir as mybir
    import concourse.tile as tile
    from concourse import bacc
    from concourse.masks import make_identity

    f32 = mybir.dt.float32
    bf16 = mybir.dt.bfloat16
    AF = mybir.ActivationFunctionType
    ALU = mybir.AluOpType
    AX = mybir.AxisListType

    # Bacc (not plain Bass): its compile() pass splits multi-semaphore waits
    # into event-semaphore sequences — TRN2 instructions encode only 1 wait.
    nc = bacc.Bacc(None)
    img_dram = {
        1: nc.declare_dram_parameter("image1", [BPC, N, D], f32, isOutput=False),
        2: nc.declare_dram_parameter("image2", [BPC, N, D], f32, isOutput=False),
    }
    out_dram = {
        1: nc.declare_dram_parameter("out1", [BPC, N, D], f32, isOutput=True),
        2: nc.declare_dram_parameter("out2", [BPC, N, D], f32, isOutput=True),
    }

    with tile.TileContext(nc) as tc:
        with (
            tc.tile_pool(name="const", bufs=1) as const_pool,
            tc.tile_pool(name="imgs", bufs=2) as imgs_pool,
            tc.tile_pool(name="work", bufs=4) as work,
            tc.tile_pool(name="outs", bufs=6) as outs,
            tc.tile_pool(name="stats", bufs=6) as stats,
            tc.tile_pool(name="spsum", bufs=2, space="PSUM") as spsum,
            tc.tile_pool(name="opsum", bufs=1, space="PSUM") as opsum,
            tc.tile_pool(name="tpsum", bufs=2, space="PSUM") as tpsum,
        ):
            ident = const_pool.tile([P, P], bf16)
            make_identity(nc, ident[:])

            imgb = {}   # (b, im) -> list of 8 natural bf16 chunk tiles
            imgT = {}   # (b, im) -> [P, DT, N] transposed bf16 tile

            def prep_loads(b):
                """Issue image loads for batch b. img1 via SWDGE cast-DMA,
                img2 via HWDGE f32 + ACT cast (parallel DMA paths)."""
                for im in (1, 2):
                    chunks = []
                    for kc in range(NT):
                        nb = imgs_pool.tile([P, D], bf16, tag=f"imgb{im}_{kc}")
                        src_ap = img_dram[im][b, kc * P : (kc + 1) * P, :]
                        if im == 1:
                            nc.gpsimd.dma_start(nb[:], src_ap)
                        else:
                            ldf = work.tile([P, D], f32, tag="ldf")
                            nc.sync.dma_start(ldf[:], src_ap)
                            nc.scalar.activation(nb[:], ldf[:], AF.Copy)
                        chunks.append(nb)
                    imgb[(b, im)] = chunks

            def prep_groups(b):
                """Return 12 closures, each PE-transposing one (im, dc) group."""
                tbs = {}
                for im in (1, 2):
                    tbs[im] = imgs_pool.tile([P, DT, N], bf16, tag=f"imgT{im}", name=f"imgT{im}")
                    imgT[(b, im)] = tbs[im]

                def make(im, dc):
                    def g():
                        chunks = imgb[(b, im)]
                        tp = tpsum.tile([P, NT, P], bf16, tag="tp")
                        for kc in range(NT):
                            nc.tensor.transpose(
                                tp[:, kc, :],
                                chunks[kc][:, dc * P : (dc + 1) * P],
                                ident[:],
                            )
                        nc.vector.tensor_copy(tbs[im][:, dc, :], tp[:])
                    return g

                return [make(im, dc) for im in (1, 2) for dc in range(DT)]

            # iteration = (batch, q_img, kv_img, q_tile); dir1 out2, dir2 out1
            iters = []
            for b in range(BPC):
                for qi in range(NT):
                    iters.append((b, 1, 2, qi))
                    iters.append((b, 2, 1, qi))
            n = len(iters)
            n0 = n // BPC  # iterations per batch

            state = {}

            def stage_a(it):
                """mm1 + softmax issue (rowmax on DVE, exp on ACT)."""
                b, q_im, kv_im, qi = it
                S = spsum.tile([P, N], f32, tag="S")
                qT = imgT[(b, q_im)]
                kT = imgT[(b, kv_im)]
                for d in range(DT):
                    lhsT = qT[:, d, qi * P : (qi + 1) * P]
                    nc.tensor.matmul(
                        S[:, :512], lhsT, kT[:, d, :512],
                        start=(d == 0), stop=(d == DT - 1),
                    )
                    nc.tensor.matmul(
                        S[:, 512:], lhsT, kT[:, d, 512:],
                        start=(d == 0), stop=(d == DT - 1),
                    )
                negmax = stats.tile([P, 1], f32, tag="negmax")
                nc.vector.tensor_reduce(
                    negmax, S[:], axis=AX.X, op=ALU.max, negate=True
                )
                Pw = work.tile([P, N], bf16, tag="P")
                nc.scalar.activation(Pw, S[:], AF.Exp, bias=negmax, scale=1.0)
                state[("P", it)] = Pw
                # prefetch the residual tile 2 slots ahead of stage_b and
                # pre-double it (DVE, off the critical path)
                b_, q_im_, kv_im_, qi_ = it
                resid = work.tile([P, D], f32, tag="resid")
                nc.sync.dma_start(
                    resid[:], img_dram[kv_im_][b_, qi_ * P : (qi_ + 1) * P, :]
                )
                resid2 = work.tile([P, D], f32, tag="resid2")
                nc.vector.tensor_scalar_mul(resid2[:], resid[:], 2.0)
                state[("R", it)] = resid2

            def stage_t(it):
                """PE-transpose P -> P^T, evacuate to SBUF via DVE."""
                Pw = state.pop(("P", it))
                tp = tpsum.tile([P, NT, P], bf16, tag="tp")
                for kc in range(NT):
                    nc.tensor.transpose(
                        tp[:, kc, :], Pw[:, kc * P : (kc + 1) * P], ident[:]
                    )
                PT = work.tile([P, NT, P], bf16, tag="PT")
                nc.vector.tensor_copy(PT[:], tp[:])
                state[("PT", it)] = PT

            def stage_b(it):
                """mm2 + normalize + residual + store."""
                b, q_im, kv_im, qi = it
                PT = state.pop(("PT", it))
                V = imgb[(b, kv_im)]
                O = opsum.tile([P, D], f32, tag="O")
                for kc in range(NT):
                    lhsT = PT[:, kc, :]
                    nc.tensor.matmul(
                        O[:, :512], lhsT, V[kc][:, :512],
                        start=(kc == 0), stop=(kc == NT - 1),
                    )
                    nc.tensor.matmul(
                        O[:, 512:], lhsT, V[kc][:, 512:],
                        start=(kc == 0), stop=(kc == NT - 1),
                    )
                # epilogue: out = l2norm(O) + 2*img_kv
                #         = O * rsqrt(sum(O^2)) + resid2   (one PSUM read of O
                # in sq, one in the fused stt -> O's buffer frees early)
                sq = work.tile([P, D], f32, tag="sq")
                ss = stats.tile([P, 1], f32, tag="ss")
                nc.scalar.activation(sq, O[:], AF.Square, accum_out=ss)
                s2 = stats.tile([P, 1], f32, tag="s2")
                nc.scalar.activation(s2, ss, AF.Sqrt)
                inv = stats.tile([P, 1], f32, tag="inv")
                nc.vector.reciprocal(inv, s2)
                resid2 = state.pop(("R", it))
                T3 = outs.tile([P, D], f32, tag="T3")
                nc.vector.scalar_tensor_tensor(
                    out=T3, in0=O[:], scalar=inv, in1=resid2[:],
                    op0=ALU.mult, op1=ALU.add,
                )
                nc.sync.dma_start(
                    out_dram[kv_im][b, qi * P : (qi + 1) * P, :], T3[:]
                )

            # batch-0 prep up front; batch b+1 loads issued 8 iters before the
            # batch boundary and its PE transposes injected into the pipeline
            # tail of batch b, where the PE would otherwise stall.
            prep_loads(0)
            for g in prep_groups(0):
                g()
            pending_groups = []
            for gi in range(n + 2):
                # stage_b first: its epilogue (DVE T1) frees the single O
                # PSUM buffer early instead of queueing behind rowmax/PTcopy
                if gi >= 2:
                    stage_b(iters[gi - 2])
                if gi < n:
                    stage_a(iters[gi])
                bidx = gi // n0 + 1  # next batch index
                if gi % n0 == n0 - 8 and bidx < BPC:
                    prep_loads(bidx)
                if gi % n0 == n0 - 4 and bidx < BPC:
                    pending_groups = prep_groups(bidx)
                if 1 <= gi <= n:
                    stage_t(iters[gi - 1])
                if pending_groups:
                    for g in pending_groups[:4]:
                        g()
                    pending_groups = pending_groups[4:]

    return nc


def _get_program():
    if "nc" not in _PROGRAM_CACHE:
        nc = build_program()
        if not nc.is_finalized():
            nc.finalize()
        _PROGRAM_CACHE["nc"] = nc
    return _PROGRAM_CACHE["nc"]


def kernel(image1: np.ndarray, image2: np.ndarray):
    from concourse.bass_utils import run_bass_kernel_spmd

    image1 = np.ascontiguousarray(image1, dtype=np.float32)
    image2 = np.ascontiguousarray(image2, dtype=np.float32)
    assert image1.shape == (B, N, D) and image2.shape == (B, N, D)

    nc = _get_program()
    core_ids = list(range(NCORES))
    in_maps = [
        {
            "image1": image1[c * BPC : (c + 1) * BPC],
            "image2": image2[c * BPC : (c + 1) * BPC],
        }
        for c in core_ids
    ]
    res = run_bass_kernel_spmd(nc, in_maps, core_ids)
    out1 = np.concatenate([res.results[c]["out1"] for c in core_ids], axis=0)
    out2 = np.concatenate([res.results[c]["out2"] for c in core_ids], axis=0)
    return out1, out2

